# revision 15
# baseline (speedup 1.0000x reference)
"""int4 weight-only quantized GEMV on 8 TRN2 NeuronCores - TensorEngine version.

out[1, n] = sum_k A[1, k] * W[n, k],   W[n,k] = (nib[n,k] - 8) * s[n,g] + z[n,g]
A: [1, 8192] fp16, B: [16384, 4096] int32 (one byte per elem, 2 nibbles),
scalesAndZeros: [16384, 256, 2] fp16 (group=32 along K).

Sharding: N=16384 rows split across 8 cores (ns=2048 each); A replicated
(baked into per-core stationaries).

Math:  out[n] = sum_g s[n,g]*dotg[n,g] + W2[n]
       dotg[n,g] = sum_{j in g} lo_j*Ae_j + hi_j*Ao_j   (j = byte col, 16/group)
       W2[n] = sum_g sA_g*(z[n,g] - 8*s[n,g])           (host, exact)

Device: host pre-expands nibbles to fp8e4 streams (lo/hi interleaved per
pair); PE fp8 DoubleRow matmuls with block-diagonal A stationaries
(A = A1 + A2/16, po rows 0-63 = A1, 64-127 = A2) accumulate group dots
into PSUM; DVE multiplies by per-virtual-group scales S2; per-q one-hot
ones stationaries reduce over groups into a single [4, 512] PSUM tile;
W2 added at the end; one output DMA.

DMA schedule: uniform ~512KB transfers (the Tile runtime keeps only ~10
DMAs in flight, so chunk size sets the prefetch byte-depth); pairs 0-11
stream as half-pair (lo/hi) transfers, pairs 12-15 as per-(pair-group,
n-quarter) chunks so the tail drains shallow; stationaries split
256KB/768KB so the first matmul waits on ~0.8MB only.
"""

import numpy as np
import ml_dtypes

import concourse.bass as bass
import concourse.bacc as bacc
import concourse.mybir as mybir
from concourse import tile
from concourse.bass_utils import run_bass_kernel_spmd

FP16 = mybir.dt.float16
FP32 = mybir.dt.float32
FP8 = mybir.dt.float8e4
Alu = mybir.AluOpType
PM = mybir.MatmulPerfMode
F8NP = ml_dtypes.float8_e4m3

M, K, N = 1, 8192, 16384
KH = K // 2            # 4096 byte-columns
GROUP = 32             # k per group -> 16 bytes per group
NG = K // GROUP        # 256 groups
NCORES = 8
NS = N // NCORES       # 2048 rows per core
P = 128
NPAIR = KH // 256      # 16 slab-pairs (256 byte-rows each)
NSB = 4                # pair-superblocks (4 pairs -> 128 virtual po rows)
NQ = NS // 512         # 4 n-chunks of 512
NHEAD = 14             # pairs 0..13 stream as full-pair DMAs


def build_program(ns=NS):
    nc = bacc.Bacc()
    # head pairs 0..13: [j, p, h, q, i, f]; 1MB full-pair DMAs (pair 0
    # split into lo/hi halves for a fast pipeline start)
    sh_d = nc.declare_dram_parameter(
        "SH", [NHEAD, P, 2, NQ, 2, 512], FP8, isOutput=False)
    # tail pairs 14,15: per-quarter chunks [q, p, tin, h, i, f] (512KB)
    st_d = nc.declare_dram_parameter(
        "ST4", [NQ, P, 2, 2, 2, 512], FP8, isOutput=False)
    sta0_d = nc.declare_dram_parameter("STA0", [P, 4, 2, 2, 128], FP8,
                                       isOutput=False)
    star_d = nc.declare_dram_parameter("STAR", [P, 3, 4, 2, 2, 128], FP8,
                                       isOutput=False)
    s2_d = nc.declare_dram_parameter("S2", [NSB, 64, ns], FP16, isOutput=False)
    w2_d = nc.declare_dram_parameter("W2", [NQ, 512], FP32, isOutput=False)
    out_d = nc.declare_dram_parameter("OUT", [NQ, 512], FP16, isOutput=True)

    with tile.TileContext(nc) as tc:
        with (
            tc.tile_pool(name="const", bufs=1) as cpool,
            tc.tile_pool(name="stream", bufs=1) as strpool,
            tc.tile_pool(name="work", bufs=6) as wpool,
            tc.tile_pool(name="ps", bufs=6, space="PSUM") as pspool,
            tc.tile_pool(name="pso", bufs=1, space="PSUM") as psopool,
        ):
            # --- constants (gpsimd: memsets + small W2 dma) ---
            one4 = cpool.tile([P, 4, 4], FP16)
            nc.gpsimd.memset(one4[:], 0.0)
            for q in range(NQ):
                nc.gpsimd.memset(one4[:, q, q : q + 1], 1.0)
            w2 = cpool.tile([NQ, 512], FP32)
            nc.gpsimd.dma_start(out=w2[:], in_=w2_d[:])

            sta0 = cpool.tile([P, 4, 2, 2, 128], FP8)
            star = cpool.tile([P, 3, 4, 2, 2, 128], FP8)
            s2f = [cpool.tile([P, ns], FP16, tag=f"s2_{sb}",
                              name=f"s2_{sb}") for sb in range(NSB)]
            sht = [strpool.tile([P, 2, NQ, 2, 512], FP8, tag=f"sh{j}",
                                name=f"sh{j}") for j in range(NHEAD)]
            st4 = [strpool.tile([P, 2, 2, 2, 512], FP8, tag=f"st{q}",
                                name=f"st{q}") for q in range(NQ)]

            def s2sl(sb, q):
                return s2f[sb][:, 512 * q : 512 * q + 512]

            # --- DMA issue: mostly 1MB units alternated across the two
            # HWDGE rings in (approximate) consumption order; the runtime
            # keeps only ~8 DMAs in flight and each completion pays ~2us
            # of receipt latency, so big uniform chunks are what keep the
            # SDMA pool saturated ---
            # three DMA rings: sync+scalar (HWDGE, 8 sem lanes) plus
            # gpsimd (SWDGE, 8 more sem lanes) -- 16 DMAs in flight keeps
            # ~8MB of prefetch queued and absorbs completion-latency jitter
            engs = [nc.sync, nc.scalar, nc.gpsimd]
            slot = [0]

            def nxt():
                e = engs[slot[0] % 3]
                slot[0] += 1
                return e

            def dma_half(j, h):
                nxt().dma_start(out=sht[j][:, h], in_=sh_d[j, :, h])

            def dma_s2(sb):
                nxt().dma_start(out=s2f[sb][0:64, :], in_=s2_d[sb])
                nc.vector.tensor_scalar(
                    out=s2f[sb][64:128, :], in0=s2f[sb][0:64, :],
                    scalar1=0.0625, scalar2=None, op0=Alu.mult)

            # front burst: all constants/scales first (the 64-partition
            # scale DMAs sag throughput wherever they sit, so confine them
            # to the ramp window), then a uniform 512KB half-pair stream
            nc.sync.dma_start(out=sta0[:], in_=sta0_d[:])
            slot[0] = 1
            dma_half(0, 0)            # scalar
            dma_half(0, 1)            # gpsimd
            for sb in range(NSB):
                dma_s2(sb)
            nxt().dma_start(out=star[:], in_=star_d[:])
            for j in range(1, NHEAD):
                dma_half(j, 0)
                dma_half(j, 1)
            for q in range(NQ):
                t_ = st4[q]
                nxt().dma_start(out=t_[:], in_=st_d[q])

            # --- compute ---
            psq = psopool.tile([4, 512], FP32, tag="psq")
            pending = []
            n_ones = [0]

            def emit_ones(sb, q, e):
                i = n_ones[0]
                n_ones[0] += 1
                nc.tensor.matmul(
                    out=psq[0:4, :], lhsT=one4[:, q], rhs=e[:, :],
                    start=(i == 0), stop=(i == NSB * NQ - 1),
                )

            def emit_tt(sb, ps):
                for q in range(NQ):
                    e = wpool.tile([P, 512], FP16, tag="e", name=f"e{sb}_{q}")
                    nc.vector.tensor_tensor(
                        out=e[:, :], in0=ps[q][:, :],
                        in1=s2sl(sb, q), op=Alu.mult,
                    )
                    pending.append((sb, q, e))

            def sta_t(sb):
                return sta0 if sb == 0 else star[:, sb - 1]

            def strip_mm(ps_q, t, c, h, rhs_ap):
                sb = t // 4
                nc.tensor.matmul(
                    out=ps_q[:, :], lhsT=sta_t(sb)[:, c if sb < 3 else t - 12, h],
                    rhs=rhs_ap, start=(c == 0 and h == 0),
                    stop=(c == 3 and h == 1),
                    perf_mode=PM.DoubleRow,
                )

            for sb in range(3):
                ps = [pspool.tile([P, 512], FP32, tag="ps", name=f"ps{sb}_{i}")
                      for i in range(NQ)]
                for c in range(4):
                    t = 4 * sb + c
                    for h in range(2):
                        for q in range(NQ):
                            strip_mm(ps[q], t, c, h, sht[t][:, h, q])
                        # flush deferred ones-matmuls (their DVE input is
                        # long done, so the PE never blocks on Vector)
                        if pending:
                            emit_ones(*pending.pop(0))
                emit_tt(sb, ps)
            # sb=3: pairs 12,13 are full-pair tiles; pairs 14,15
            # arrive as per-quarter chunks (q-major tail)
            ps = [pspool.tile([P, 512], FP32, tag="ps", name=f"ps3_{i}")
                  for i in range(NQ)]
            for c in range(2):
                for h in range(2):
                    for q in range(NQ):
                        strip_mm(ps[q], 12 + c, c, h, sht[12 + c][:, h, q])
                    if pending:
                        emit_ones(*pending.pop(0))
            # (pairs 14,15 continue the same accumulation via the chunk tiles)
            for q in range(NQ):
                for tin in range(2):
                    for h in range(2):
                        strip_mm(ps[q], 14 + tin, 2 + tin, h,
                                 st4[q][:, tin, h])
                if pending:
                    emit_ones(*pending.pop(0))
            emit_tt(3, ps)
            for item in pending:
                emit_ones(*item)

            outt = wpool.tile([NQ, 512], FP16, tag="outt")
            nc.vector.tensor_tensor(
                out=outt[:, :], in0=psq[0:4, :], in1=w2[:, :], op=Alu.add,
            )
            nc.sync.dma_start(out=out_d[:], in_=outt[:, :])
    nc.finalize()
    return nc


_NC_CACHE = {}


def _get_program(ns=NS):
    if ns not in _NC_CACHE:
        _NC_CACHE[ns] = build_program(ns)
    return _NC_CACHE[ns]


def _split_fp8(c):
    """c (fp32 array) -> (A1, A2) fp8 with c ~ A1 + A2/16."""
    a1 = c.astype(F8NP)
    resid = (c - a1.astype(np.float32)) * 16.0
    a2 = resid.astype(F8NP)
    return a1, a2


def prep_inputs(A, B, scalesAndZeros):
    """Host prep: nibble->fp8 streams, stationaries, scales, W2."""
    A = np.asarray(A).reshape(K).astype(np.float32)
    B = np.asarray(B)
    SZ = np.asarray(scalesAndZeros)

    # fp8 nibble LUT expansion, transposed to [KH, N]
    b8 = B.astype(np.uint8)              # [N, KH]
    lut = np.arange(16, dtype=np.float32).astype(F8NP)  # exact
    lo8 = lut[b8 & 15]                   # [N, KH] fp8
    hi8 = lut[b8 >> 4]
    lo8_t = np.ascontiguousarray(lo8.T)  # [KH, N]
    hi8_t = np.ascontiguousarray(hi8.T)

    # stationaries: per byte-row kb: lo coef Ae=A[2kb], hi coef Ao=A[2kb+1]
    ae = A[0::2]
    ao = A[1::2]
    ae1, ae2 = _split_fp8(ae)
    ao1, ao2 = _split_fp8(ao)
    sta = np.zeros((P, NPAIR, 2, 2, 128), F8NP)
    kb = np.arange(KH)
    tt, ii, pp, uu = kb // 256, (kb // 128) % 2, kb % 128, (kb // 16) % 16
    band = tt % 4  # po: A1 rows 0..63 (16*band+u), A2 rows 64..127
    sta[pp, tt, 0, ii, 16 * band + uu] = ae1[kb]
    sta[pp, tt, 0, ii, 64 + 16 * band + uu] = ae2[kb]
    sta[pp, tt, 1, ii, 16 * band + uu] = ao1[kb]
    sta[pp, tt, 1, ii, 64 + 16 * band + uu] = ao2[kb]
    sta_r = sta.reshape(P, NSB, 4, 2, 2, 128)
    sta0 = np.ascontiguousarray(sta_r[:, 0])
    star = np.ascontiguousarray(sta_r[:, 1:])

    s = SZ[..., 0].astype(np.float32)    # [N, NG]
    z = SZ[..., 1].astype(np.float32)
    sag = A.reshape(NG, GROUP).sum(-1, dtype=np.float64).astype(np.float32)
    w2_full = (sag[None, :] * (z - 8.0 * s)).sum(-1, dtype=np.float64).astype(np.float32)

    s2_full = np.zeros((NSB, 64, N), np.float16)
    for sb in range(NSB):
        for c in range(4):
            g0 = 16 * (4 * sb + c)
            s_blk = s[:, g0 : g0 + 16].T          # [16, N]
            s2_full[sb, 16 * c : 16 * c + 16] = s_blk.astype(np.float16)

    # streams, all cores at once:
    # X[h, t, i, p, core, q, f] with kb = 256t + 128i + p, n = 2048c + 512q + f
    X = np.stack([lo8_t, hi8_t]).reshape(2, NPAIR, 2, P, NCORES, NQ, 512)
    # head pairs 0..13 [core, j, p, h, q, i, f]
    sh_all = np.empty((NCORES, NHEAD, P, 2, NQ, 2, 512), F8NP)
    sh_all[...] = X[:, :NHEAD].transpose(4, 1, 3, 0, 5, 2, 6)
    # tail pairs 14,15 [core, q, p, tin, h, i, f]
    st_all = np.empty((NCORES, NQ, P, 2, 2, 2, 512), F8NP)
    st_all[...] = X[:, NHEAD:].transpose(4, 5, 3, 1, 0, 2, 6)
    s2_cores = np.ascontiguousarray(
        s2_full.reshape(NSB, 64, NCORES, NS).transpose(2, 0, 1, 3))

    in_maps = []
    for core in range(NCORES):
        n0, n1 = core * NS, (core + 1) * NS
        in_maps.append({
            "SH": sh_all[core],
            "ST4": st_all[core],
            "S2": s2_cores[core],
            "STA0": sta0,
            "STAR": star,
            "W2": np.ascontiguousarray(w2_full[n0:n1].reshape(NQ, 512)),
        })
    return in_maps


def kernel(A, B, scalesAndZeros):
    in_maps = prep_inputs(A, B, scalesAndZeros)
    nc = _get_program()
    res = run_bass_kernel_spmd(nc, in_maps, core_ids=list(range(NCORES)))
    out = np.concatenate([res.results[c]["OUT"].reshape(NS) for c in range(NCORES)])
    return out.reshape(1, N).astype(np.float16)


if __name__ == "__main__":
    rng = np.random.default_rng(0)
    A = rng.standard_normal((M, K)).astype(np.float16)
    B = rng.integers(0, 256, (N, KH)).astype(np.int32)
    SZ = rng.standard_normal((N, NG, 2)).astype(np.float16)
    out = kernel(A, B, SZ)
    bb = B.astype(np.int64)
    q = np.stack([bb & 15, (bb >> 4) & 15], axis=-1).reshape(N, K).astype(np.float64) - 8.0
    s = SZ[..., 0].astype(np.float64)
    z = SZ[..., 1].astype(np.float64)
    W = (q.reshape(N, NG, GROUP) * s[:, :, None] + z[:, :, None]).reshape(N, K)
    exp = (A.astype(np.float64) @ W.T).astype(np.float16)
    err = np.abs(out.astype(np.float64) - exp.astype(np.float64))
    rel = err / np.maximum(np.abs(exp.astype(np.float64)), 1e-6)
    print("median rel:", np.median(rel), "absmax/scale:",
          err.max() / np.abs(exp).max())


# revision 16
# speedup vs baseline: 1.0662x; 1.0662x over previous
"""int4 weight-only quantized GEMV on 8 TRN2 NeuronCores - TensorEngine version.

out[1, n] = sum_k A[1, k] * W[n, k],   W[n,k] = (nib[n,k] - 8) * s[n,g] + z[n,g]
A: [1, 8192] fp16, B: [16384, 4096] int32 (one byte per elem, 2 nibbles),
scalesAndZeros: [16384, 256, 2] fp16 (group=32 along K).

Sharding: N=16384 rows split across 8 cores (ns=2048 each); A replicated
(baked into per-core stationaries).

Math:  out[n] = sum_g s[n,g]*dotg[n,g] + W2[n]
       dotg[n,g] = sum_{j in g} lo_j*Ae_j + hi_j*Ao_j   (j = byte col, 16/group)
       W2[n] = sum_g sA_g*(z[n,g] - 8*s[n,g])           (host, exact)

Device: host pre-expands nibbles to fp8e4 streams (lo/hi interleaved per
pair); PE fp8 DoubleRow matmuls with block-diagonal A stationaries
(A = A1 + A2/16, po rows 0-63 = A1, 64-127 = A2) accumulate group dots
into PSUM; DVE multiplies by per-virtual-group scales S2; per-q one-hot
ones stationaries reduce over groups into a single [4, 512] PSUM tile;
W2 added at the end; one output DMA.

DMA schedule: uniform ~512KB transfers (the Tile runtime keeps only ~10
DMAs in flight, so chunk size sets the prefetch byte-depth); pairs 0-11
stream as half-pair (lo/hi) transfers, pairs 12-15 as per-(pair-group,
n-quarter) chunks so the tail drains shallow; stationaries split
256KB/768KB so the first matmul waits on ~0.8MB only.
"""

import numpy as np
import ml_dtypes

import concourse.bass as bass
import concourse.bacc as bacc
import concourse.mybir as mybir
from concourse import tile
from concourse.bass_utils import run_bass_kernel_spmd

FP16 = mybir.dt.float16
FP32 = mybir.dt.float32
FP8 = mybir.dt.float8e4
Alu = mybir.AluOpType
PM = mybir.MatmulPerfMode
F8NP = ml_dtypes.float8_e4m3

M, K, N = 1, 8192, 16384
KH = K // 2            # 4096 byte-columns
GROUP = 32             # k per group -> 16 bytes per group
NG = K // GROUP        # 256 groups
NCORES = 8
NS = N // NCORES       # 2048 rows per core
P = 128
NPAIR = KH // 256      # 16 slab-pairs (256 byte-rows each)
NSB = 4                # pair-superblocks (4 pairs -> 128 virtual po rows)
NQ = NS // 512         # 4 n-chunks of 512
NHEAD = 14             # pairs 0..13 stream as full-pair DMAs


def build_program(ns=NS):
    nc = bacc.Bacc()
    # head pairs 0..13: [j, p, h, q, i, f]; 1MB full-pair DMAs (pair 0
    # split into lo/hi halves for a fast pipeline start)
    sh_d = nc.declare_dram_parameter(
        "SH", [NHEAD, P, 2, NQ, 2, 512], FP8, isOutput=False)
    # tail pairs 14,15: per-quarter chunks [q, p, tin, h, i, f] (512KB)
    st_d = nc.declare_dram_parameter(
        "ST4", [NQ, P, 2, 2, 2, 512], FP8, isOutput=False)
    sta0_d = nc.declare_dram_parameter("STA0", [P, 4, 2, 2, 128], FP8,
                                       isOutput=False)
    star_d = nc.declare_dram_parameter("STAR", [P, 3, 4, 2, 2, 128], FP8,
                                       isOutput=False)
    s2_d = nc.declare_dram_parameter("S2", [NSB, P, ns], FP16, isOutput=False)
    w2_d = nc.declare_dram_parameter("W2", [NQ, 512], FP32, isOutput=False)
    out_d = nc.declare_dram_parameter("OUT", [NQ, 512], FP16, isOutput=True)

    with tile.TileContext(nc) as tc:
        with (
            tc.tile_pool(name="const", bufs=1) as cpool,
            tc.tile_pool(name="stream", bufs=1) as strpool,
            tc.tile_pool(name="work", bufs=6) as wpool,
            tc.tile_pool(name="ps", bufs=6, space="PSUM") as pspool,
            tc.tile_pool(name="pso", bufs=1, space="PSUM") as psopool,
        ):
            # --- constants (gpsimd: memsets + small W2 dma) ---
            one4 = cpool.tile([P, 4, 4], FP16)
            nc.gpsimd.memset(one4[:], 0.0)
            for q in range(NQ):
                nc.gpsimd.memset(one4[:, q, q : q + 1], 1.0)
            w2 = cpool.tile([NQ, 512], FP32)
            nc.gpsimd.dma_start(out=w2[:], in_=w2_d[:])

            sta0 = cpool.tile([P, 4, 2, 2, 128], FP8)
            star = cpool.tile([P, 3, 4, 2, 2, 128], FP8)
            s2f = [cpool.tile([P, ns], FP16, tag=f"s2_{sb}",
                              name=f"s2_{sb}") for sb in range(NSB)]
            sht = [strpool.tile([P, 2, NQ, 2, 512], FP8, tag=f"sh{j}",
                                name=f"sh{j}") for j in range(NHEAD)]
            st4 = [strpool.tile([P, 2, 2, 2, 512], FP8, tag=f"st{q}",
                                name=f"st{q}") for q in range(NQ)]

            def s2sl(sb, q):
                return s2f[sb][:, 512 * q : 512 * q + 512]

            # --- DMA issue: mostly 1MB units alternated across the two
            # HWDGE rings in (approximate) consumption order; the runtime
            # keeps only ~8 DMAs in flight and each completion pays ~2us
            # of receipt latency, so big uniform chunks are what keep the
            # SDMA pool saturated ---
            engs = [nc.sync, nc.scalar]
            slot = [0]

            def nxt():
                e = engs[slot[0] & 1]
                slot[0] += 1
                return e

            def dma_half(j, h):
                nxt().dma_start(out=sht[j][:, h], in_=sh_d[j, :, h])

            def dma_s2(sb):
                nxt().dma_start(out=s2f[sb][:], in_=s2_d[sb])

            # front burst: all constants/scales first (the 64-partition
            # scale DMAs sag throughput wherever they sit, so confine them
            # to the ramp window), then a uniform 512KB half-pair stream
            nc.sync.dma_start(out=sta0[:], in_=sta0_d[:])
            slot[0] = 1
            dma_half(0, 0)            # scalar
            dma_half(0, 1)            # sync
            for sb in range(NSB):
                dma_s2(sb)
            nxt().dma_start(out=star[:], in_=star_d[:])
            for j in range(1, NHEAD):
                dma_half(j, 0)
                dma_half(j, 1)
            for q in range(NQ):
                t_ = st4[q]
                nxt().dma_start(out=t_[:], in_=st_d[q])

            # --- compute ---
            psq = psopool.tile([4, 512], FP32, tag="psq")
            pending = []
            n_ones = [0]

            def emit_ones(sb, q, e):
                i = n_ones[0]
                n_ones[0] += 1
                nc.tensor.matmul(
                    out=psq[0:4, :], lhsT=one4[:, q], rhs=e[:, :],
                    start=(i == 0), stop=(i == NSB * NQ - 1),
                )

            def emit_tt(sb, ps):
                for q in range(NQ):
                    e = wpool.tile([P, 512], FP16, tag="e", name=f"e{sb}_{q}")
                    nc.vector.tensor_tensor(
                        out=e[:, :], in0=ps[q][:, :],
                        in1=s2sl(sb, q), op=Alu.mult,
                    )
                    pending.append((sb, q, e))

            def sta_t(sb):
                return sta0 if sb == 0 else star[:, sb - 1]

            def strip_mm(ps_q, t, c, h, rhs_ap):
                sb = t // 4
                nc.tensor.matmul(
                    out=ps_q[:, :], lhsT=sta_t(sb)[:, c if sb < 3 else t - 12, h],
                    rhs=rhs_ap, start=(c == 0 and h == 0),
                    stop=(c == 3 and h == 1),
                    perf_mode=PM.DoubleRow,
                )

            for sb in range(3):
                ps = [pspool.tile([P, 512], FP32, tag="ps", name=f"ps{sb}_{i}")
                      for i in range(NQ)]
                for c in range(4):
                    t = 4 * sb + c
                    for h in range(2):
                        for q in range(NQ):
                            strip_mm(ps[q], t, c, h, sht[t][:, h, q])
                        # flush deferred ones-matmuls (their DVE input is
                        # long done, so the PE never blocks on Vector)
                        if pending:
                            emit_ones(*pending.pop(0))
                emit_tt(sb, ps)
            # sb=3: pairs 12,13 are full-pair tiles; pairs 14,15
            # arrive as per-quarter chunks (q-major tail)
            ps = [pspool.tile([P, 512], FP32, tag="ps", name=f"ps3_{i}")
                  for i in range(NQ)]
            for c in range(2):
                for h in range(2):
                    for q in range(NQ):
                        strip_mm(ps[q], 12 + c, c, h, sht[12 + c][:, h, q])
                    if pending:
                        emit_ones(*pending.pop(0))
            # (pairs 14,15 continue the same accumulation via the chunk tiles)
            for q in range(NQ):
                for tin in range(2):
                    for h in range(2):
                        strip_mm(ps[q], 14 + tin, 2 + tin, h,
                                 st4[q][:, tin, h])
                if pending:
                    emit_ones(*pending.pop(0))
            emit_tt(3, ps)
            for item in pending:
                emit_ones(*item)

            outt = wpool.tile([NQ, 512], FP16, tag="outt")
            nc.vector.tensor_tensor(
                out=outt[:, :], in0=psq[0:4, :], in1=w2[:, :], op=Alu.add,
            )
            nc.sync.dma_start(out=out_d[:], in_=outt[:, :])
    nc.finalize()
    return nc


_NC_CACHE = {}


def _get_program(ns=NS):
    if ns not in _NC_CACHE:
        _NC_CACHE[ns] = build_program(ns)
    return _NC_CACHE[ns]


def _split_fp8(c):
    """c (fp32 array) -> (A1, A2) fp8 with c ~ A1 + A2/16."""
    a1 = c.astype(F8NP)
    resid = (c - a1.astype(np.float32)) * 16.0
    a2 = resid.astype(F8NP)
    return a1, a2


def prep_inputs(A, B, scalesAndZeros):
    """Host prep: nibble->fp8 streams, stationaries, scales, W2."""
    A = np.asarray(A).reshape(K).astype(np.float32)
    B = np.asarray(B)
    SZ = np.asarray(scalesAndZeros)

    # fp8 nibble LUT expansion, transposed to [KH, N]
    b8 = B.astype(np.uint8)              # [N, KH]
    lut = np.arange(16, dtype=np.float32).astype(F8NP)  # exact
    lo8 = lut[b8 & 15]                   # [N, KH] fp8
    hi8 = lut[b8 >> 4]
    lo8_t = np.ascontiguousarray(lo8.T)  # [KH, N]
    hi8_t = np.ascontiguousarray(hi8.T)

    # stationaries: per byte-row kb: lo coef Ae=A[2kb], hi coef Ao=A[2kb+1]
    ae = A[0::2]
    ao = A[1::2]
    ae1, ae2 = _split_fp8(ae)
    ao1, ao2 = _split_fp8(ao)
    sta = np.zeros((P, NPAIR, 2, 2, 128), F8NP)
    kb = np.arange(KH)
    tt, ii, pp, uu = kb // 256, (kb // 128) % 2, kb % 128, (kb // 16) % 16
    band = tt % 4  # po: A1 rows 0..63 (16*band+u), A2 rows 64..127
    sta[pp, tt, 0, ii, 16 * band + uu] = ae1[kb]
    sta[pp, tt, 0, ii, 64 + 16 * band + uu] = ae2[kb]
    sta[pp, tt, 1, ii, 16 * band + uu] = ao1[kb]
    sta[pp, tt, 1, ii, 64 + 16 * band + uu] = ao2[kb]
    sta_r = sta.reshape(P, NSB, 4, 2, 2, 128)
    sta0 = np.ascontiguousarray(sta_r[:, 0])
    star = np.ascontiguousarray(sta_r[:, 1:])

    s = SZ[..., 0].astype(np.float32)    # [N, NG]
    z = SZ[..., 1].astype(np.float32)
    sag = A.reshape(NG, GROUP).sum(-1, dtype=np.float64).astype(np.float32)
    w2_full = (sag[None, :] * (z - 8.0 * s)).sum(-1, dtype=np.float64).astype(np.float32)

    # full 128-row scale tiles: rows 0:64 = A1 scales, 64:128 = /16
    # (host-expanded so every scale DMA is a uniform full-partition unit)
    s2_full = np.zeros((NSB, P, N), np.float16)
    for sb in range(NSB):
        for c in range(4):
            g0 = 16 * (4 * sb + c)
            s_blk = s[:, g0 : g0 + 16].T          # [16, N]
            s2_full[sb, 16 * c : 16 * c + 16] = s_blk.astype(np.float16)
            s2_full[sb, 64 + 16 * c : 64 + 16 * c + 16] = (
                s_blk * 0.0625).astype(np.float16)

    # streams, all cores at once:
    # X[h, t, i, p, core, q, f] with kb = 256t + 128i + p, n = 2048c + 512q + f
    X = np.stack([lo8_t, hi8_t]).reshape(2, NPAIR, 2, P, NCORES, NQ, 512)
    # head pairs 0..13 [core, j, p, h, q, i, f]
    sh_all = np.empty((NCORES, NHEAD, P, 2, NQ, 2, 512), F8NP)
    sh_all[...] = X[:, :NHEAD].transpose(4, 1, 3, 0, 5, 2, 6)
    # tail pairs 14,15 [core, q, p, tin, h, i, f]
    st_all = np.empty((NCORES, NQ, P, 2, 2, 2, 512), F8NP)
    st_all[...] = X[:, NHEAD:].transpose(4, 5, 3, 1, 0, 2, 6)
    s2_cores = np.ascontiguousarray(
        s2_full.reshape(NSB, P, NCORES, NS).transpose(2, 0, 1, 3))

    in_maps = []
    for core in range(NCORES):
        n0, n1 = core * NS, (core + 1) * NS
        in_maps.append({
            "SH": sh_all[core],
            "ST4": st_all[core],
            "S2": s2_cores[core],
            "STA0": sta0,
            "STAR": star,
            "W2": np.ascontiguousarray(w2_full[n0:n1].reshape(NQ, 512)),
        })
    return in_maps


def kernel(A, B, scalesAndZeros):
    in_maps = prep_inputs(A, B, scalesAndZeros)
    nc = _get_program()
    res = run_bass_kernel_spmd(nc, in_maps, core_ids=list(range(NCORES)))
    out = np.concatenate([res.results[c]["OUT"].reshape(NS) for c in range(NCORES)])
    return out.reshape(1, N).astype(np.float16)


if __name__ == "__main__":
    rng = np.random.default_rng(0)
    A = rng.standard_normal((M, K)).astype(np.float16)
    B = rng.integers(0, 256, (N, KH)).astype(np.int32)
    SZ = rng.standard_normal((N, NG, 2)).astype(np.float16)
    out = kernel(A, B, SZ)
    bb = B.astype(np.int64)
    q = np.stack([bb & 15, (bb >> 4) & 15], axis=-1).reshape(N, K).astype(np.float64) - 8.0
    s = SZ[..., 0].astype(np.float64)
    z = SZ[..., 1].astype(np.float64)
    W = (q.reshape(N, NG, GROUP) * s[:, :, None] + z[:, :, None]).reshape(N, K)
    exp = (A.astype(np.float64) @ W.T).astype(np.float16)
    err = np.abs(out.astype(np.float64) - exp.astype(np.float64))
    rel = err / np.maximum(np.abs(exp.astype(np.float64)), 1e-6)
    print("median rel:", np.median(rel), "absmax/scale:",
          err.max() / np.abs(exp).max())
